# revision 76
# baseline (speedup 1.0000x reference)
"""Trainium2 Bass kernel for nn_MLP_Route_RL_Model (route RL model).

Reference math (per batch element b of 256):
  - state = [route_nums (48) | customers (48*24*36)]
  - customer MLP (tanh-tanh, 36->128->32) on every node of every route
  - 2-layer GRU (hidden 128) over the 24 nodes of each of the 48 routes
  - route summary mean, node-selection MLP 256->256->128->24, masked softmax

Sharding: pure data parallel over batch B=256 -> 8 cores x 32.

v3 layout notes:
  - activations feature-major ([feature, token]); matmuls contract over the
    partition dim, tokens on the free dim (chunks of 512 = one PSUM bank).
  - r,z gates accumulate into one 2-bank PSUM tile -> ONE fused sigmoid per
    chunk; GRU biases enter via bias-row matmuls (ones-row trick) and are
    skipped entirely when zero (the spec'd case).
  - n-gate: t = r*gh_n on DVE; t is accumulated onto gi_n in PSUM with an
    identity matmul (PE), so tanh reads PSUM directly - saves a DVE add.
  - two interleaved streams l0(t) and l1(t-1), op-interleaved so every
    engine FIFO always has ready work; their tanh inputs share one 2-bank
    pair tile so a single ACT op covers both streams.
  - gating h' = n + z*(h-n): h-n on gpsimd (otherwise idle), rest on DVE
    fp16-SBUF tensor_tensor (2x mode).
  - weights/consts arrive in two packed blob DMAs; customer features in six
    per-group DMAs.
"""

import sys

import numpy as np

sys.path.insert(0, "/opt/trn_rl_repo")

import concourse.bass as bass  # noqa: E402
import concourse.bacc as bacc  # noqa: E402
import concourse.mybir as mybir  # noqa: E402
import concourse.tile as tile  # noqa: E402
from concourse.bass_utils import run_bass_kernel_spmd  # noqa: E402

F32 = mybir.dt.float32
F16 = mybir.dt.float16
AF = mybir.ActivationFunctionType
OP = mybir.AluOpType

# Problem shape constants
B = 256
NCORES = 8
BLOC = B // NCORES          # 32 batch rows per core
MR = 48                     # routes per batch
MN = 24                     # nodes per route
FEAT = 36
CH = 128                    # customer hidden
CO = 32                     # customer out
GH = 128                    # GRU hidden
S = BLOC * MR               # sequences per core = 1536
NC = 512                    # token chunk (PSUM bank width in fp32)
NCH = S // NC               # chunks per core = 3
NG = MN // 4                # node groups of 4 (cust_out partition stacking)
QT = S // 128               # 128-token tiles per core = 12

# f16 blob column offsets
_off = 0
def _seg(n):
    global _off
    o = _off
    _off += n
    return o
O_WC1 = _seg(CH)
O_WC2 = _seg(CO)
O_WIH0 = _seg(3 * GH)
O_WHH0 = _seg(3 * GH)
O_WIH1 = _seg(3 * GH)
O_WHH1 = _seg(3 * GH)
O_IDENT = _seg(128)
O_WN1A = _seg(256)
O_WN1B = _seg(256)
O_WN2A = _seg(128)
O_WN2B = _seg(128)
O_WN3 = _seg(MN)
O_SEL = _seg(S)
O_ONES = _seg(NC)           # row-0 ones for bias-row matmuls
O_BROW = _seg(6 * GH)       # row-0 bias rows: l0 r,z,in then l1 r,z,in
F16COLS = _off

# f32 blob column offsets
_off = 0
O_BC1 = _seg(1)
O_BC2 = _seg(1)
O_BHN0 = _seg(1)
O_BHN1 = _seg(1)
O_BN1 = _seg(2)
O_BN2 = _seg(1)
O_IOTA = _seg(MN)
O_ONES128 = _seg(128)       # row-0 [1,128] ones
O_BN3 = _seg(MN)            # row-0 [1,24]
F32COLS = _off

_cache = {}


def _build(rz_zero=True, bin_zero=True, hn_zero=True, bn1_zero=True):
    """Trace + schedule the per-core Tile kernel. Returns the Bass module."""
    nc = bacc.Bacc("TRN2", target_bir_lowering=False, debug=False)

    # ---- DRAM I/O ----------------------------------------------------------
    d_cust = nc.dram_tensor("cust_fm", [FEAT, MN * S], F16, kind="ExternalInput")
    d_rn = nc.dram_tensor("rn12", [128, QT], F32, kind="ExternalInput")
    d_b16 = nc.dram_tensor("blob16", [128, F16COLS], F16, kind="ExternalInput")
    d_b32 = nc.dram_tensor("blob32", [128, F32COLS], F32, kind="ExternalInput")
    d_out = nc.dram_tensor("out_tm", [QT, 128, MN], F32, kind="ExternalOutput")

    with tile.TileContext(nc) as tc:
        with (
            tc.tile_pool(name="wpool", bufs=1) as wp,
            tc.tile_pool(name="state", bufs=1) as sp,
        ):
            # phase-A weights (wc1, wc2) first so the MLP can start while the
            # larger remainder of the blob still streams in
            b16 = wp.tile([128, F16COLS], F16, tag="b16", name="b16")
            nc.sync.dma_start(b16[:, 0:O_WIH0], d_b16.ap()[:, 0:O_WIH0])
            b32 = wp.tile([128, F32COLS], F32, tag="b32", name="b32")
            nc.sync.dma_start(b32[:], d_b32.ap())
            # the big blob half is issued inside phase A, after the first
            # customer-feature DMA - its wire time must not delay group 0
            rn12 = wp.tile([128, QT], F32, tag="rn12", name="rn12")

            wc1 = b16[0:FEAT, O_WC1:O_WC1 + CH]
            wc2 = b16[0:CH, O_WC2:O_WC2 + CO]
            wih = {0: b16[:, O_WIH0:O_WIH0 + 3 * GH],
                   1: b16[0:GH, O_WIH1:O_WIH1 + 3 * GH]}
            whh = {0: b16[0:GH, O_WHH0:O_WHH0 + 3 * GH],
                   1: b16[0:GH, O_WHH1:O_WHH1 + 3 * GH]}
            ident = b16[:, O_IDENT:O_IDENT + 128]
            wn1a = b16[0:GH, O_WN1A:O_WN1A + 256]
            wn1b = b16[0:GH, O_WN1B:O_WN1B + 256]
            wn2a = b16[:, O_WN2A:O_WN2A + 128]
            wn2b = b16[:, O_WN2B:O_WN2B + 128]
            wn3 = b16[0:GH, O_WN3:O_WN3 + MN]
            sel = b16[0:BLOC, O_SEL:O_SEL + S]
            ones512 = b16[0:1, O_ONES:O_ONES + NC]
            brow = {(l, g): b16[0:1, O_BROW + (3 * l + gi) * GH:
                                O_BROW + (3 * l + gi + 1) * GH]
                    for l in (0, 1) for gi, g in enumerate(("r", "z", "in"))}
            bc1 = b32[0:CH, O_BC1:O_BC1 + 1]
            bc2s = b32[:, O_BC2:O_BC2 + 1]
            bhn = {0: b32[0:GH, O_BHN0:O_BHN0 + 1],
                   1: b32[0:GH, O_BHN1:O_BHN1 + 1]}
            bn1 = b32[:, O_BN1:O_BN1 + 2]
            bn2 = b32[:, O_BN2:O_BN2 + 1]
            iota24 = b32[:, O_IOTA:O_IOTA + MN]
            ones128 = b32[0:1, O_ONES128:O_ONES128 + 128]
            bn3r = b32[0:1, O_BN3:O_BN3 + MN]

            # persistent state: customer-MLP output, GRU hidden states
            # cust layout: partition = (n%4)*32 + f, free = (n//4)*S + s
            cust = sp.tile([128, NG * S], F16, tag="cust_out", name="cust")
            hh = sp.tile([GH, 2 * S], F16, tag="hh", name="hh")

            # touch every activation function once while the blobs stream in,
            # so the ACT table loads never land mid-phase (warm memset first -
            # the hh memset would delay it in the DVE queue)
            warm = sp.tile([128, 1], F32, tag="warm", name="warm")
            nc.vector.memset(warm[:], 0.0)
            for fn_ in (AF.Exp, AF.Relu, AF.Tanh, AF.Sigmoid):
                nc.scalar.activation(warm[:], warm[:], fn_)
            nc.vector.memset(hh[:], 0.0)

            # ---- phase A: customer MLP (tanh-tanh 36->128->32) -------------
            with (
                tc.tile_pool(name="xin", bufs=3) as xp,
                tc.tile_pool(name="h1c", bufs=3) as h1p,
                tc.tile_pool(name="psA", bufs=2, space="PSUM") as psA,
                tc.tile_pool(name="psA2", bufs=1, space="PSUM") as psA2,
            ):
                # software-pipelined: node-pair p's first matmuls are emitted
                # before pair p-1's second matmuls so PE never waits on ACT.
                work = []           # stage-1-done items awaiting stage 2
                done = {}
                c2ps = {}

                def _flushA(item):
                    g, c2t, sb, kp, h1q = item
                    for jj in range(2):
                        k = 2 * kp + jj
                        nc.tensor.matmul(
                            c2t[32 * k:32 * (k + 1), sb * NC:(sb + 1) * NC],
                            wc2, h1q[:, jj * NC:(jj + 1) * NC],
                            tile_position=(0, 32 * k),
                        )
                    done[g] = done.get(g, 0) + 1
                    if done[g] == 6:
                        # whole group accumulated: one tanh over [128, S]
                        nc.scalar.activation(
                            cust[:, g * S:(g + 1) * S], c2t[:],
                            AF.Tanh, bias=bc2s,
                        )

                for g in range(NG):
                    xg = xp.tile([FEAT, 4 * S], F16, tag="xg", name=f"xg{g}")
                    if g == 0:
                        # split so the first node pair lands sooner
                        nc.sync.dma_start(xg[:, 0:2 * S],
                                          d_cust.ap()[:, 0:2 * S])
                        nc.sync.dma_start(xg[:, 2 * S:4 * S],
                                          d_cust.ap()[:, 2 * S:4 * S])
                    else:
                        nc.sync.dma_start(xg[:], d_cust.ap()[:, 4 * g * S:
                                                             4 * (g + 1) * S])
                    if g == 1:
                        # big blob half is only needed at GRU start (~60us);
                        # keep its wire time off the group-1 feature DMA
                        nc.sync.dma_start(b16[:, O_WIH0:],
                                          d_b16.ap()[:, O_WIH0:])
                    c2t = psA2.tile([128, S], F32, tag="c2t",
                                    name=f"c2_{g}")
                    for sb in range(NCH):
                        for kp in range(2):         # node pairs in the group
                            p1p = psA.tile([128, 2 * NC], F32, tag="p1p",
                                           name=f"p1_{g}_{sb}_{kp}")
                            for jj in range(2):
                                k = 2 * kp + jj
                                nc.tensor.matmul(
                                    p1p[:, jj * NC:(jj + 1) * NC], wc1,
                                    xg[:, k * S + sb * NC:
                                       k * S + (sb + 1) * NC],
                                )
                            h1q = h1p.tile([CH, 2 * NC], F16, tag="h1q",
                                           name=f"h1q_{g}_{sb}_{kp}")
                            nc.scalar.activation(h1q[:], p1p[:], AF.Tanh,
                                                 bias=bc1)
                            work.append((g, c2t, sb, kp, h1q))
                            if len(work) >= 2:
                                _flushA(work.pop(0))
                while work:
                    _flushA(work.pop(0))

            # ---- phase B: 2-layer GRU over MN steps ------------------------
            with (
                tc.tile_pool(name="gact", bufs=6) as gp,
                tc.tile_pool(name="gupd", bufs=6) as up,
                tc.tile_pool(name="przp", bufs=2, space="PSUM") as przp,
                tc.tile_pool(name="pairp", bufs=2, space="PSUM") as pairp,
            ):
                hhv = hh[:].rearrange("p (l s) -> p l s", l=2)

                class Cell:
                    """Matmul emission for one GRU cell-chunk; the elementwise
                    work runs pair-wide at the iteration level."""

                    def __init__(self, layer, t, c, half):
                        self.layer, self.t, self.c = layer, t, c
                        self.half = half            # slice of the pair tiles
                        c0, c1 = c * NC, (c + 1) * NC
                        self.has_h = t > 0
                        self.h = hh[:, layer * S + c0: layer * S + c1]
                        self.w_h = whh[layer]
                        w_x = wih[layer]
                        if layer == 0:
                            g, k4 = t // 4, t % 4
                            p0 = 32 * k4
                            self.xap = cust[p0:p0 + CO, g * S + c0: g * S + c1]
                            self.wx = w_x[p0:p0 + CO, :]
                            self.tp = (p0, 0)
                        else:
                            self.xap = hh[:, c0:c1]
                            self.wx = w_x
                            self.tp = (0, 0)
                        self.nm = f"{layer}_{t}_{c}"

                    def rz_mm(self):
                        prz = przp.tile([128, 2 * NC], F32, tag="prz",
                                        name=f"prz{self.nm}")
                        pr, pz = prz[:, 0:NC], prz[:, NC:2 * NC]
                        if self.has_h:
                            nc.tensor.matmul(pr, self.w_h[:, 0:GH], self.h,
                                             start=True, stop=False)
                            nc.tensor.matmul(pr, self.wx[:, 0:GH], self.xap,
                                             start=False, stop=rz_zero,
                                             tile_position=self.tp)
                            nc.tensor.matmul(pz, self.w_h[:, GH:2 * GH], self.h,
                                             start=True, stop=False)
                            nc.tensor.matmul(pz, self.wx[:, GH:2 * GH], self.xap,
                                             start=False, stop=rz_zero,
                                             tile_position=self.tp)
                        else:
                            nc.tensor.matmul(pr, self.wx[:, 0:GH], self.xap,
                                             start=True, stop=rz_zero,
                                             tile_position=self.tp)
                            nc.tensor.matmul(pz, self.wx[:, GH:2 * GH], self.xap,
                                             start=True, stop=rz_zero,
                                             tile_position=self.tp)
                        if not rz_zero:
                            lay = self.layer
                            nc.tensor.matmul(pr, brow[(lay, "r")], ones512,
                                             start=False, stop=True)
                            nc.tensor.matmul(pz, brow[(lay, "z")], ones512,
                                             start=False, stop=True)
                        self.prz = prz

                    def sig(self, rzpair):
                        # write r into rzpair[:, half] and z into the matching
                        # slot 2*NC further - one ACT op, 2-segment output AP
                        rzv = rzpair[:].rearrange("p (z g s) -> p z g s",
                                                  z=2, g=2)
                        nc.scalar.activation(rzv[:, :, self.half, :],
                                             self.prz[:], AF.Sigmoid)

                    def ph_mm(self, php):
                        if not self.has_h:
                            return
                        nc.tensor.matmul(php[:, self.half * NC:
                                             (self.half + 1) * NC],
                                         self.w_h[:, 2 * GH:3 * GH], self.h)

                    def pi_mm(self, pip, t_):
                        pi = pip[:, self.half * NC:(self.half + 1) * NC]
                        last = bin_zero and not self.has_h
                        nc.tensor.matmul(pi, self.wx[:, 2 * GH:3 * GH],
                                         self.xap, start=True, stop=last,
                                         tile_position=self.tp)
                        if not bin_zero:
                            nc.tensor.matmul(pi, brow[(self.layer, "in")],
                                             ones512, start=False,
                                             stop=not self.has_h)
                        if self.has_h:
                            nc.tensor.matmul(
                                pi, ident,
                                t_[:, self.half * NC:(self.half + 1) * NC],
                                start=False, stop=True)

                class Iter:
                    """One pipeline iteration: paired cells + pair tiles."""

                    def __init__(self, t, c, k):
                        self.k, self.c = k, c
                        self.cells = []
                        if t < MN:
                            self.cells.append(Cell(0, t, c, 0))
                        if t > 0:
                            self.cells.append(Cell(1, t - 1, c,
                                                   len(self.cells)))
                        self.n = len(self.cells)
                        c0, c1 = c * NC, (c + 1) * NC
                        if self.n == 2:
                            self.hseg = hhv[:, :, c0:c1]
                        else:
                            lay = self.cells[0].layer
                            self.hseg = hhv[:, lay:lay + 1, c0:c1]
                        self.nth = sum(1 for x in self.cells if x.has_h)

                def front(it):
                    for x in it.cells:
                        x.rz_mm()
                    rzpair = gp.tile([128, 4 * NC], F16, tag="rz",
                                     name=f"rz{it.k}")
                    it.rzpair = rzpair
                    for x in it.cells:
                        x.sig(rzpair)
                    php = pairp.tile([128, 2 * NC], F32, tag="hp",
                                     name=f"php{it.k}")
                    for x in it.cells:
                        x.ph_mm(php)
                    # t = r * gh_n, both streams in one op (cells with h
                    # always occupy the leading halves)
                    w = it.nth * NC
                    it.t_ = gp.tile([GH, 2 * NC], F16, tag="t_",
                                    name=f"t{it.k}")
                    if w:
                        if hn_zero:
                            nc.vector.tensor_mul(it.t_[:, 0:w], php[:, 0:w],
                                                 rzpair[:, 0:w])
                        else:
                            for x in it.cells:
                                if not x.has_h:
                                    continue
                                sl = slice(x.half * NC, (x.half + 1) * NC)
                                nc.vector.scalar_tensor_tensor(
                                    it.t_[:, sl], php[:, sl], bhn[x.layer],
                                    rzpair[:, sl], OP.add, OP.mult)
                    # zm = z - 1 and u = z*h use only the sigmoid and OLD h -
                    # off the recurrence chain
                    wn = it.n * NC
                    it.zm = up.tile([GH, 2 * NC], F16, tag="zm",
                                    name=f"zm{it.k}")
                    nc.vector.tensor_scalar(it.zm[:, 0:wn],
                                            rzpair[:, 2 * NC:2 * NC + wn],
                                            1.0, None, OP.subtract)
                    it.u_ = up.tile([GH, 2 * NC], F16, tag="u_",
                                    name=f"u{it.k}")
                    for x in it.cells:
                        sl = slice(x.half * NC, (x.half + 1) * NC)
                        nc.gpsimd.tensor_mul(
                            it.u_[:, sl],
                            rzpair[:, 2 * NC + x.half * NC:
                                   2 * NC + (x.half + 1) * NC], x.h)

                def back(it):
                    wn = it.n * NC
                    pip = pairp.tile([128, 2 * NC], F32, tag="hp",
                                     name=f"pip{it.k}")
                    for x in it.cells:
                        x.pi_mm(pip, it.t_)
                    np_ = gp.tile([GH, 2 * NC], F16, tag="np", name=f"np{it.k}")
                    nc.scalar.activation(np_[:, 0:wn], pip[:, 0:wn], AF.Tanh)
                    # y = (z-1)*n ; h' = u - y = z*h + (1-z)*n
                    # per-stream y and h': the l0 recurrence doesn't wait on
                    # the l1 half
                    y_ = up.tile([GH, 2 * NC], F16, tag="y_", name=f"y{it.k}")
                    for x in it.cells:
                        sl = slice(x.half * NC, (x.half + 1) * NC)
                        nc.vector.tensor_mul(y_[:, sl], it.zm[:, sl], np_[:, sl])
                        nc.vector.tensor_sub(x.h, it.u_[:, sl], y_[:, sl])

                pend = None
                for t in range(MN):
                    for c in range(NCH):
                        it = Iter(t, c, t * NCH + c)
                        if pend is not None:
                            back(pend)
                        front(it)
                        pend = it
                # final step (l1(MN-1)) = three mutually independent
                # chunk-cells: batch their fronts so ACT runs the sigmoids
                # back-to-back with no recurrence stagger
                back(pend)
                tail = [Iter(MN, c, MN * NCH + c) for c in range(NCH)]
                front(tail[0])
                front(tail[1])
                back(tail[0])
                front(tail[2])
                back(tail[1])
                back(tail[2])

                # ---- phase C: route mean + node MLP + masked softmax -------
                # emitted inside the GRU pool scope (reusing its PSUM tags) so
                # it overlaps the recurrence drain.
                with tc.tile_pool(name="fin", bufs=4) as fp_:
                    po_all = fp_.tile([128, QT * MN], F32, tag="po_all",
                                      name="po_all")
                    mean32 = fp_.tile([GH, BLOC], F32, tag="mean32",
                                      name="mean32")
                    h2 = hh[:, S:2 * S]
                    rn12_d = nc.sync.dma_start(rn12[:], d_rn.ap())
                    # h2-side matmuls of the first two chunks run during the
                    # mean-reduce chain (they don't depend on it)
                    p1ps = {}
                    for c in range(2):
                        c0, c1 = c * NC, (c + 1) * NC
                        p1ps[c] = pairp.tile([128, 2 * NC], F32, tag="hp",
                                             name=f"p1_{c}")
                        for m in range(2):
                            nc.tensor.matmul(
                                p1ps[c][:, m * NC:(m + 1) * NC],
                                wn1a[:, 128 * m:128 * (m + 1)],
                                h2[:, c0:c1], start=True, stop=False)
                    mean = fp_.tile([GH, BLOC], F16, tag="mean", name="mean")
                    # summing 48 f16 routes; wn1b is prescaled by 1/48 so the
                    # magnitude stays O(1) - f16 accumulation is ample here.
                    # Batch-aligned partial reduces per chunk (+ the two
                    # straddler batches) so most of the reduction runs as soon
                    # as each h2 chunk goes final, overlapping the GRU tail.
                    def rmean(b0, b1):
                        seg = h2[:, b0 * MR:b1 * MR].rearrange(
                            "p (b r) -> p b r", r=MR)
                        nc.vector.tensor_reduce(mean[:, b0:b1], seg,
                                                mybir.AxisListType.X, OP.add)
                    with nc.allow_low_precision(reason="route-sum of 48 f16 "
                                                "values, final rel tol 2e-2"):
                        rmean(0, 10)        # tokens 0..480     (chunk 0)
                        rmean(11, 21)       # tokens 528..1008  (chunk 1)
                        rmean(10, 11)       # straddles chunks 0/1
                        rmean(22, 32)       # tokens 1056..1536 (chunk 2)
                        rmean(21, 22)       # straddles chunks 1/2
                    pmt = przp.tile([128, 2 * NC], F32, tag="prz", name="pmt")
                    nc.tensor.matmul(pmt[0:BLOC, 0:256], mean[:], wn1b)
                    mmt = fp_.tile([BLOC, 256], F16, tag="mmt", name="mmt")
                    nc.vector.tensor_copy(mmt[:], pmt[0:BLOC, 0:256])

                    # all masks depend only on rn12 - emit them upfront
                    mskt = fp_.tile([128, QT * MN], F32, tag="mskt",
                                    name="mskt")
                    for qi in range(QT):
                        nc.vector.tensor_scalar(
                            mskt[:, qi * MN:(qi + 1) * MN], iota24,
                            rn12[:, qi:qi + 1], None, OP.is_lt)

                    def cfront(c):
                        c0, c1 = c * NC, (c + 1) * NC
                        if c in p1ps:
                            p1p = p1ps[c]
                            for m in range(2):
                                nc.tensor.matmul(
                                    p1p[:, m * NC:(m + 1) * NC],
                                    mmt[:, 128 * m:128 * (m + 1)],
                                    sel[:, c0:c1], start=False, stop=bn1_zero)
                        else:
                            p1p = pairp.tile([128, 2 * NC], F32, tag="hp",
                                             name=f"p1_{c}")
                            for m in range(2):
                                p1 = p1p[:, m * NC:(m + 1) * NC]
                                nc.tensor.matmul(
                                    p1, wn1a[:, 128 * m:128 * (m + 1)],
                                    h2[:, c0:c1], start=True, stop=False)
                                nc.tensor.matmul(
                                    p1, mmt[:, 128 * m:128 * (m + 1)],
                                    sel[:, c0:c1], start=False, stop=bn1_zero)
                        # a1-relu on DVE (tensor_scalar max) - measured best
                        # placement.  (NOT gpsimd: the Pool engine has no
                        # PSUM port.)
                        a1 = fp_.tile([128, 2 * NC], F16, tag="a1",
                                      name=f"a1_{c}")
                        if bn1_zero:
                            nc.vector.tensor_scalar(a1[:], p1p[:], 0.0, None,
                                                    OP.max)
                        else:
                            for m in range(2):
                                sl = slice(m * NC, (m + 1) * NC)
                                nc.vector.tensor_scalar(
                                    a1[:, sl], p1p[:, sl], bn1[:, m:m + 1],
                                    0.0, OP.add, OP.max)
                        return a1

                    def cback(c, a1):
                        p2q = przp.tile([128, 2 * NC], F32, tag="prz",
                                        name=f"p2_{c}")
                        p2 = p2q[:, 0:NC]
                        nc.tensor.matmul(p2, wn2a, a1[:, 0:NC],
                                         start=True, stop=False)
                        nc.tensor.matmul(p2, wn2b, a1[:, NC:2 * NC],
                                         start=False, stop=True)
                        n2 = fp_.tile([128, NC], F16, tag="n2", name=f"n2_{c}")
                        nc.vector.tensor_scalar(n2[:], p2, bn2, 0.0,
                                                OP.add, OP.max)
                        plqt = przp.tile([128, 2 * NC], F32, tag="prz",
                                         name=f"plq_{c}")
                        plq = plqt[:, 0:4 * MN]
                        for q in range(4):
                            pl = plqt[:, q * MN:(q + 1) * MN]
                            nc.tensor.matmul(pl, n2[:, q * 128:(q + 1) * 128],
                                             wn3, start=True, stop=False)
                            nc.tensor.matmul(pl, ones128, bn3r,
                                             start=False, stop=True)
                        ex = fp_.tile([128, 4 * MN], F32, tag="ex",
                                      name=f"ex_{c}")
                        nc.scalar.activation(ex[:], plq, AF.Exp)
                        sm = fp_.tile([128, 4], F32, tag="sm", name=f"sm_{c}")
                        exv = ex[:].rearrange("p (q j) -> p q j", j=MN)
                        nc.vector.tensor_reduce(sm[:], exv,
                                                mybir.AxisListType.X, OP.add)
                        rec = fp_.tile([128, 4], F32, tag="rec",
                                       name=f"rec_{c}")
                        nc.vector.reciprocal(rec[:], sm[:])
                        for q in range(4):
                            qi = 4 * c + q
                            nc.vector.scalar_tensor_tensor(
                                po_all[:, qi * MN:(qi + 1) * MN],
                                ex[:, q * MN:(q + 1) * MN],
                                rec[:, q:q + 1],
                                mskt[:, qi * MN:(qi + 1) * MN],
                                OP.mult, OP.mult)
                        # store this chunk's slice immediately - the final DMA
                        # then only covers the last chunk
                        nc.sync.dma_start(
                            d_out.ap()[4 * c:4 * (c + 1)].rearrange(
                                "q p j -> p q j"),
                            po_all[:, 4 * c * MN:4 * (c + 1) * MN].rearrange(
                                "p (q j) -> p q j", j=MN))

                    cpend = None
                    for c in range(NCH):
                        a1 = cfront(c)
                        if cpend is not None:
                            cback(*cpend)
                        cpend = (c, a1)
                    cback(*cpend)


    nc.compile()
    return nc


def _prep_inputs(inputs):
    """Host-side preprocessing -> (per-core input dicts, build flags)."""
    state = np.ascontiguousarray(inputs["state"], dtype=np.float32)
    rn = state[:, :MR]                                    # [B, 48]
    cust = state[:, MR:].reshape(B, MR, MN, FEAT)

    def f32(x):
        return np.ascontiguousarray(np.asarray(x, dtype=np.float32))

    Wih0 = f32(inputs["Wih0"]); Whh0 = f32(inputs["Whh0"])
    Wih1 = f32(inputs["Wih1"]); Whh1 = f32(inputs["Whh1"])
    bih0 = f32(inputs["bih0"]); bhh0 = f32(inputs["bhh0"])
    bih1 = f32(inputs["bih1"]); bhh1 = f32(inputs["bhh1"])
    bn1v = f32(inputs["bn1"])

    rz_zero = not any(np.any(b[:2 * GH]) for b in (bih0, bhh0, bih1, bhh1))
    bin_zero = not (np.any(bih0[2 * GH:]) or np.any(bih1[2 * GH:]))
    hn_zero = not (np.any(bhh0[2 * GH:]) or np.any(bhh1[2 * GH:]))
    bn1_zero = not np.any(bn1v)

    b16 = np.zeros((128, F16COLS), np.float16)
    def put16(off, arr, rows=None):
        a = np.asarray(arr, np.float16)
        r = a.shape[0] if rows is None else rows
        b16[0:r, off:off + a.shape[1]] = a
    put16(O_WC1, np.asarray(inputs["Wc1"], np.float16))
    put16(O_WC2, np.asarray(inputs["Wc2"], np.float16))
    put16(O_WIH0, np.tile(Wih0.astype(np.float16), (4, 1)))
    put16(O_WHH0, Whh0.astype(np.float16))
    put16(O_WIH1, Wih1.astype(np.float16))
    put16(O_WHH1, Whh1.astype(np.float16))
    put16(O_IDENT, np.eye(128, dtype=np.float16))
    put16(O_WN1A, f32(inputs["Wn1"])[0:GH, :].astype(np.float16))
    put16(O_WN1B, (f32(inputs["Wn1"])[GH:, :] / np.float32(MR)).astype(np.float16))
    put16(O_WN2A, f32(inputs["Wn2"])[0:128, :].astype(np.float16))
    put16(O_WN2B, f32(inputs["Wn2"])[128:256, :].astype(np.float16))
    put16(O_WN3, np.asarray(inputs["Wn3"], np.float16))
    selm = np.zeros((BLOC, S), np.float32)
    selm[np.arange(S) // MR, np.arange(S)] = 1.0
    put16(O_SEL, selm.astype(np.float16))
    b16[0, O_ONES:O_ONES + NC] = 1.0
    for l, (bi, bh) in enumerate(((bih0, bhh0), (bih1, bhh1))):
        b16[0, O_BROW + 3 * l * GH:O_BROW + (3 * l + 1) * GH] = \
            (bi[0:GH] + bh[0:GH]).astype(np.float16)
        b16[0, O_BROW + (3 * l + 1) * GH:O_BROW + (3 * l + 2) * GH] = \
            (bi[GH:2 * GH] + bh[GH:2 * GH]).astype(np.float16)
        b16[0, O_BROW + (3 * l + 2) * GH:O_BROW + (3 * l + 3) * GH] = \
            bi[2 * GH:].astype(np.float16)

    b32 = np.zeros((128, F32COLS), np.float32)
    b32[0:CH, O_BC1] = f32(inputs["bc1"])
    b32[:, O_BC2] = np.tile(f32(inputs["bc2"]).reshape(CO), 4)
    b32[0:GH, O_BHN0] = bhh0[2 * GH:]
    b32[0:GH, O_BHN1] = bhh1[2 * GH:]
    b32[:, O_BN1:O_BN1 + 2] = bn1v.reshape(2, 128).T
    b32[:, O_BN2] = f32(inputs["bn2"])
    b32[:, O_IOTA:O_IOTA + MN] = np.arange(MN, dtype=np.float32)[None, :]
    b32[0, O_ONES128:O_ONES128 + 128] = 1.0
    b32[0, O_BN3:O_BN3 + MN] = f32(inputs["bn3"])

    com = {"blob16": b16, "blob32": b32}

    in_maps = []
    for core in range(NCORES):
        b0, b1 = core * BLOC, (core + 1) * BLOC
        # cust_fm[f, n*S + (b*MR+r)] = cust[b, r, n, f]
        cfm = cust[b0:b1].transpose(3, 2, 0, 1).reshape(FEAT, MN * S)
        m = dict(com)
        m["cust_fm"] = np.ascontiguousarray(cfm.astype(np.float16))
        rcore = rn[b0:b1].reshape(S)
        m["rn12"] = np.ascontiguousarray(rcore.reshape(QT, 128).T)
        in_maps.append(m)
    return in_maps, (rz_zero, bin_zero, hn_zero, bn1_zero)


def _run(inputs, **kw):
    in_maps, flags = _prep_inputs(inputs)
    key = ("nc",) + flags
    if key not in _cache:
        _cache[key] = _build(*flags)
        _cache["nc"] = _cache[key]
    nc = _cache[key]
    return run_bass_kernel_spmd(nc, in_maps, core_ids=list(range(NCORES)), **kw)


def kernel(**inputs) -> np.ndarray:
    res = _run(inputs)
    # out_tm is [QT, 128, MN]; token t = q*128 + p, so a plain reshape
    # recovers [S, MN] per core.
    outs = [r["out_tm"].reshape(S, MN) for r in res.results]
    return np.concatenate(outs, axis=0).reshape(B, MR, MN)
